# revision 31
# baseline (speedup 1.0000x reference)
"""Multi-head attention (B=4, L=2048, D=1024, H=16) on 8 trn2 NeuronCores.

Sharding: core c -> batch b = c//2, head half hh = c%2 (8 heads / 512 proj
columns per core).  Host pre-transposes per-batch inputs to [D, L] (cast to
fp16) so all device matmuls have their contraction dim on partitions; Wo
partial products of the two half-head cores of each batch are summed on the
host during the final unshard (each core adds bo * 0.5 so the host sum
restores bo once).

Device pipeline per core (fp16 matmuls at N=1024 -> full PE rate + FWL;
PSUM accumulation, softmax, attention probabilities and outputs in fp32):
  1. Projections: QT/KT in [dk, tokens] fp16 layout (lhsT = W tile), V in
     [tokens, dk] fp16 layout interleaved per head with a ones column so
     the P@V matmul also produces softmax row sums for free.
  2. Per head, two q-half sweeps: phase A computes S^T = K^T.T@Q^T per
     k-tile (fp32 PSUM), exp on ACT (fp16 out), and accumulates ctx^T
     (plus sums row) in PSUM; softmax row sums are transposed to column
     layout with tiny PE transposes and reciprocated.  Phase B recomputes
     S in natural [q, k] layout, exp to fp32, and DVE scales rows by the
     reciprocal sums to produce attention rows, DMAd out.
  3. ctx^T tiles are normalized once at the end (reciprocal rows broadcast
     back from a DRAM spill) and projected through Wo rows.
"""

import os
import sys

import numpy as np

for _p in ("/root/.axon_site/_ro/trn_rl_repo", "/opt/trn_rl_repo"):
    if os.path.isdir(_p) and _p not in sys.path:
        sys.path.append(_p)

import concourse.bacc as bacc_mod
import concourse.bass as bass
import concourse.mybir as mybir
from concourse.bass_utils import run_bass_kernel_spmd
from concourse.tile import TileContext

B, L, D, H, DK = 4, 2048, 1024, 16, 64
NCORES = 8
HPC = H // 2  # heads per core
DHC = HPC * DK  # proj columns per core (512)
P = 128
VW = HPC * (DK + 1)  # V tile width incl. ones columns (520)
FP32 = mybir.dt.float32
FP16 = mybir.dt.float16
AF = mybir.ActivationFunctionType
ALU = mybir.AluOpType

N_KT = D // P  # 8   k-tiles over d_model in projections
N_MT = DHC // P  # 4   dk-col tiles of QT/KT
N_TT = L // P  # 16  token tiles
NCH = 512  # matmul moving chunk (one fp32 PSUM bank)
HCH = 1024  # q/k half width

LAST_RESULT = {}


def _ensure_ntff_hook():
    """Register the axon NTFF profile hook if the antenv stub lacks it."""
    import types

    try:
        import antenv.axon_hooks  # noqa: F401

        return
    except ImportError:
        pass
    m = types.ModuleType("antenv.axon_hooks")
    hook_box = [None]
    m.set_axon_ntff_profile_hook = lambda h: hook_box.__setitem__(0, h)
    m.get_axon_ntff_profile_hook = lambda: hook_box[0]
    sys.modules["antenv.axon_hooks"] = m
    import antenv

    antenv.axon_hooks = m
    try:
        from trn_agent_boot.trn_boot import _ntff_profile_via_ctypes

        hook = _ntff_profile_via_ctypes("/opt/axon/libaxon_pjrt.so")
        if hook is not None:
            m.set_axon_ntff_profile_hook(hook)
    except Exception as e:
        print("ntff hook setup failed:", e)


def _build() -> bass.Bass:
    nc = bacc_mod.Bacc()

    xq_t = nc.dram_tensor("xq_t", [D, L], FP16, kind="ExternalInput")
    xk_t = nc.dram_tensor("xk_t", [D, L], FP16, kind="ExternalInput")
    xv_t = nc.dram_tensor("xv_t", [D, L], FP16, kind="ExternalInput")
    wq = nc.dram_tensor("wq", [D, DHC], FP16, kind="ExternalInput")
    wk = nc.dram_tensor("wk", [D, DHC], FP16, kind="ExternalInput")
    wv = nc.dram_tensor("wv", [D, DHC], FP16, kind="ExternalInput")
    wo = nc.dram_tensor("wo", [DHC, D], FP16, kind="ExternalInput")
    bq = nc.dram_tensor("bq", [1, DHC], FP32, kind="ExternalInput")
    bk = nc.dram_tensor("bk", [1, DHC], FP32, kind="ExternalInput")
    bv = nc.dram_tensor("bv", [1, DHC], FP32, kind="ExternalInput")
    bo = nc.dram_tensor("bo", [1, D], FP32, kind="ExternalInput")
    attn_out = nc.dram_tensor("attn_out", [HPC, L, L], FP32, kind="ExternalOutput")
    out_part = nc.dram_tensor("out_part", [L, D], FP32, kind="ExternalOutput")
    sum_spill = nc.dram_tensor("sum_spill", [HPC, L], FP32)

    with TileContext(nc) as tc:
        with (
            tc.tile_pool(name="qt", bufs=N_MT) as qt_pool,
            tc.tile_pool(name="kt", bufs=N_MT) as kt_pool,
            tc.tile_pool(name="vt", bufs=N_TT) as vt_pool,
            tc.tile_pool(name="cxt", bufs=N_MT) as cxt_pool,
            tc.tile_pool(name="small", bufs=1) as small_pool,
            tc.tile_pool(name="hsm", bufs=2) as hsm_pool,
            tc.tile_pool(name="ps", bufs=3, space="PSUM") as ps_pool,
            tc.tile_pool(name="ctx", bufs=1, space="PSUM") as ctx_pool,
        ):
            neg_col = small_pool.tile([P, 1], FP32)
            nc.vector.memset(neg_col, -1.0)

            # per-partition bias columns for QT/KT: col m holds b[m*128:(m+1)*128]
            bq_cols = small_pool.tile([P, N_MT], FP32)
            bk_cols = small_pool.tile([P, N_MT], FP32)
            nc.sync.dma_start(bq_cols, bq.rearrange("1 (m p) -> p m", p=P))
            nc.sync.dma_start(bk_cols, bk.rearrange("1 (m p) -> p m", p=P))
            bv_bcast = small_pool.tile([P, DHC], FP32)
            _src = bv[:, :]
            nc.gpsimd.dma_start(
                bv_bcast,
                bass.AP(tensor=_src.tensor, offset=_src.offset, ap=[[0, P], [1, DHC]]),
            )
            bo_bcast = small_pool.tile([P, D], FP32)
            _src = bo[:, :]
            nc.gpsimd.dma_start(
                bo_bcast,
                bass.AP(tensor=_src.tensor, offset=_src.offset, ap=[[0, P], [1, D]]),
            )

            qt_tiles = [
                qt_pool.tile([P, L], FP16, name=f"qt{i}", tag="qt")
                for i in range(N_MT)
            ]
            kt_tiles = [
                kt_pool.tile([P, L], FP16, name=f"kt{i}", tag="kt")
                for i in range(N_MT)
            ]
            vt_tiles = [
                vt_pool.tile([P, VW], FP16, name=f"vt{i}", tag="vt")
                for i in range(N_TT)
            ]

            # ---------------- projections ----------------
            with (
                tc.tile_pool(name="xt", bufs=N_KT + 1) as xt_pool,
                tc.tile_pool(name="w", bufs=N_KT + 1) as w_pool,
            ):
                for x_dram, w_dram, mode in (
                    (xq_t, wq, "QT"),
                    (xk_t, wk, "KT"),
                    (xv_t, wv, "V"),
                ):
                    xts = []
                    ws = []
                    for kt in range(N_KT):
                        xtile = xt_pool.tile(
                            [P, L], FP16, name=f"x_{mode}{kt}", tag="xt"
                        )
                        nc.sync.dma_start(xtile, x_dram[kt * P : (kt + 1) * P, :])
                        xts.append(xtile)
                        wtile = w_pool.tile(
                            [P, DHC], FP16, name=f"w_{mode}{kt}", tag="w"
                        )
                        nc.sync.dma_start(wtile, w_dram[kt * P : (kt + 1) * P, :])
                        ws.append(wtile)

                    if mode in ("QT", "KT"):
                        dst = qt_tiles if mode == "QT" else kt_tiles
                        bcols = bq_cols if mode == "QT" else bk_cols
                        for m in range(N_MT):
                            for q in range(L // NCH):
                                ps = ps_pool.tile([P, NCH], FP32, name="ps", tag="s")
                                for kt in range(N_KT):
                                    nc.tensor.matmul(
                                        ps,
                                        lhsT=ws[kt][:, m * P : (m + 1) * P],
                                        rhs=xts[kt][:, q * NCH : (q + 1) * NCH],
                                        start=(kt == 0),
                                        stop=(kt == N_KT - 1),
                                    )
                                nc.vector.tensor_scalar_add(
                                    dst[m][:, q * NCH : (q + 1) * NCH],
                                    ps,
                                    bcols[:, m : m + 1],
                                )
                    else:
                        for mt in range(N_TT):
                            ps = ps_pool.tile([P, DHC], FP32, name="ps", tag="s")
                            for kt in range(N_KT):
                                nc.tensor.matmul(
                                    ps,
                                    lhsT=xts[kt][:, mt * P : (mt + 1) * P],
                                    rhs=ws[kt],
                                    start=(kt == 0),
                                    stop=(kt == N_KT - 1),
                                )
                            vt = vt_tiles[mt]
                            vt_r = vt.rearrange("p (h c) -> p h c", c=DK + 1)
                            nc.vector.memset(vt_r[:, :, DK : DK + 1], 1.0)
                            nc.vector.tensor_tensor(
                                vt_r[:, :, 0:DK],
                                ps.rearrange("p (h c) -> p h c", c=DK),
                                bv_bcast.rearrange("p (h c) -> p h c", c=DK),
                                op=ALU.add,
                            )

            # ---------------- attention ----------------
            cxt_tiles = [
                cxt_pool.tile([P, L], FP16, name=f"cxt{i}", tag="cxt")
                for i in range(N_MT)
            ]
            with (
                tc.tile_pool(name="pt", bufs=4) as pt_pool,
                tc.tile_pool(name="pb", bufs=4) as pb_pool,
                tc.tile_pool(name="stg", bufs=2) as stg_pool,
                tc.tile_pool(name="attn", bufs=5) as attn_pool,
                tc.tile_pool(name="wo", bufs=N_MT) as wo_pool,
                tc.tile_pool(name="out", bufs=2) as out_pool,
            ):
                wo_tiles = []
                for ct in range(N_MT):
                    wtile = wo_pool.tile([P, D], FP16, name=f"wo{ct}", tag="wo")
                    nc.sync.dma_start(wtile, wo[ct * P : (ct + 1) * P, :])
                    wo_tiles.append(wtile)

                for h in range(HPC):
                    th, r0 = h // 2, (h % 2) * DK
                    qth = qt_tiles[th][r0 : r0 + DK, :]
                    kth = kt_tiles[th][r0 : r0 + DK, :]

                    # ---- phase A: S^T tiles, exp, ctx^T (+sums) accumulation
                    rcol = hsm_pool.tile([P, N_TT], FP32, name="rcol", tag="rcol")
                    for half in range(2):
                        q0 = half * HCH
                        ctx = ctx_pool.tile([P, HCH], FP32, name="ctx", tag="ctx")
                        for kt in range(N_TT):
                            vh = vt_tiles[kt][:, h * (DK + 1) : (h + 1) * (DK + 1)]
                            st = ps_pool.tile([P, HCH], FP32, name="st", tag="s")
                            for c in range(2):
                                nc.tensor.matmul(
                                    st[:, c * NCH : (c + 1) * NCH],
                                    lhsT=kth[:, kt * P : (kt + 1) * P],
                                    rhs=qth[:, q0 + c * NCH : q0 + (c + 1) * NCH],
                                    start=True,
                                    stop=True,
                                )
                            pt = pt_pool.tile([P, HCH], FP16, name="pt", tag="pt")
                            nc.scalar.activation(pt, st, AF.Exp, scale=0.125)
                            for c in range(2):
                                nc.tensor.matmul(
                                    ctx[0 : DK + 1, c * NCH : (c + 1) * NCH],
                                    lhsT=vh,
                                    rhs=pt[:, c * NCH : (c + 1) * NCH],
                                    start=(kt == 0),
                                    stop=(kt == N_TT - 1),
                                )

                        # stage ctx out of PSUM fast; recip cols via PE transposes
                        stg = stg_pool.tile([P, HCH], FP32, name="stg", tag="stg")
                        nc.vector.tensor_copy(
                            stg[0 : DK + 1, :], ctx[0 : DK + 1, :]
                        )
                        nc.sync.dma_start(
                            sum_spill[h : h + 1, q0 : q0 + HCH],
                            stg[DK : DK + 1, :],
                        )
                        scp = ctx_pool.tile([P, 8], FP32, name="scp", tag="ctx")
                        for j in range(8):
                            nc.tensor.transpose(
                                scp[:, j : j + 1],
                                stg[DK : DK + 1, j * P : (j + 1) * P],
                                neg_col[DK : DK + 1, 0:1],
                            )
                        nc.vector.reciprocal(rcol[:, half * 8 : (half + 1) * 8], scp)
                        # unnormalized fp16 ctx^T copy releases stg
                        nc.vector.tensor_copy(
                            cxt_tiles[th][r0 : r0 + DK, q0 : q0 + HCH],
                            stg[0:DK, :],
                        )

                    # ---- phase B: natural S, exp fp32, row-scale, DMA out
                    for qt in range(N_TT):
                        at = attn_pool.tile([P, L], FP32, name="at", tag="at")
                        for half in range(2):
                            st = ps_pool.tile([P, HCH], FP32, name="st", tag="s")
                            for c in range(2):
                                ks = half * HCH + c * NCH
                                nc.tensor.matmul(
                                    st[:, c * NCH : (c + 1) * NCH],
                                    lhsT=qth[:, qt * P : (qt + 1) * P],
                                    rhs=kth[:, ks : ks + NCH],
                                    start=True,
                                    stop=True,
                                )
                            p = pb_pool.tile([P, HCH], FP32, name="p", tag="pb")
                            nc.scalar.activation(p, st, AF.Exp, scale=0.125)
                            nc.gpsimd.tensor_scalar_mul(
                                at[:, half * HCH : (half + 1) * HCH],
                                p,
                                rcol[:, qt : qt + 1],
                            )
                        nc.sync.dma_start(attn_out[h, qt * P : (qt + 1) * P, :], at)

                # ---- normalize ctx^T (all heads) now that recips are spilled
                for ct in range(N_MT):
                    recb = hsm_pool.tile([P, L], FP32, name="recb", tag="recb")
                    for sub in range(2):
                        hh = 2 * ct + sub
                        _src = sum_spill[hh : hh + 1, :]
                        nc.gpsimd.dma_start(
                            recb[sub * DK : (sub + 1) * DK, :],
                            bass.AP(
                                tensor=_src.tensor,
                                offset=_src.offset,
                                ap=[[0, DK], [1, L]],
                            ),
                        )
                    nc.vector.reciprocal(recb, recb)
                    nc.vector.tensor_tensor(
                        cxt_tiles[ct], cxt_tiles[ct], recb, op=ALU.mult
                    )

                # ---------------- output projection ----------------
                for qt in range(N_TT):
                    ot = out_pool.tile([P, D], FP32, name="ot", tag="ot")
                    ps = ps_pool.tile([P, D], FP32, name="ps", tag="s")
                    for n2 in range(2):
                        for ct in range(N_MT):
                            nc.tensor.matmul(
                                ps[:, n2 * NCH : (n2 + 1) * NCH],
                                lhsT=cxt_tiles[ct][:, qt * P : (qt + 1) * P],
                                rhs=wo_tiles[ct][:, n2 * NCH : (n2 + 1) * NCH],
                                start=(ct == 0),
                                stop=(ct == N_MT - 1),
                            )
                    nc.vector.tensor_add(ot, ps, bo_bcast)
                    nc.sync.dma_start(out_part[qt * P : (qt + 1) * P, :], ot)

    nc.compile()
    return nc


_NC_CACHE = []


def kernel(query, key_, value, Wq, bq, Wk, bk, Wv, bv, Wo, bo, trace=False):
    query = np.asarray(query, np.float32)
    key_ = np.asarray(key_, np.float32)
    value = np.asarray(value, np.float32)
    Wq, Wk, Wv, Wo = (np.asarray(a, np.float32) for a in (Wq, Wk, Wv, Wo))
    bq, bk, bv, bo = (np.asarray(a, np.float32) for a in (bq, bk, bv, bo))

    if not _NC_CACHE:
        _NC_CACHE.append(_build())
    nc = _NC_CACHE[0]

    in_maps = []
    for c in range(NCORES):
        b, s = c // 2, (c % 2) * DHC
        in_maps.append(
            {
                "xq_t": np.ascontiguousarray(query[b].T.astype(np.float16)),
                "xk_t": np.ascontiguousarray(key_[b].T.astype(np.float16)),
                "xv_t": np.ascontiguousarray(value[b].T.astype(np.float16)),
                "wq": np.ascontiguousarray(Wq[:, s : s + DHC].astype(np.float16)),
                "wk": np.ascontiguousarray(Wk[:, s : s + DHC].astype(np.float16)),
                "wv": np.ascontiguousarray(Wv[:, s : s + DHC].astype(np.float16)),
                "wo": np.ascontiguousarray(Wo[s : s + DHC, :].astype(np.float16)),
                "bq": np.ascontiguousarray(bq.reshape(1, -1)[:, s : s + DHC]),
                "bk": np.ascontiguousarray(bk.reshape(1, -1)[:, s : s + DHC]),
                "bv": np.ascontiguousarray(bv.reshape(1, -1)[:, s : s + DHC]),
                "bo": np.ascontiguousarray(bo.reshape(1, -1)) * 0.5,
            }
        )

    if trace:
        _ensure_ntff_hook()
    res = run_bass_kernel_spmd(nc, in_maps, list(range(NCORES)), trace=trace)
    LAST_RESULT["exec_time_ns"] = res.exec_time_ns
    LAST_RESULT["trace"] = res.instructions_and_trace

    attn = np.empty((B, H, L, L), np.float32)
    out = np.empty((B, L, D), np.float32)
    for c in range(NCORES):
        b, h0 = c // 2, (c % 2) * HPC
        attn[b, h0 : h0 + HPC] = res.results[c]["attn_out"]
    for b in range(B):
        out[b] = res.results[2 * b]["out_part"] + res.results[2 * b + 1]["out_part"]
    return out, attn


# revision 32
# speedup vs baseline: 5.0310x; 5.0310x over previous
"""Multi-head attention (B=4, L=2048, D=1024, H=16) on 8 trn2 NeuronCores.

Sharding: core c -> batch b = c//2, head half hh = c%2 (8 heads / 512 proj
columns per core).  Host pre-transposes per-batch inputs to [D, L] (cast to
fp16) so all device matmuls have their contraction dim on partitions; Wo
partial products of the two half-head cores of each batch are summed on the
host during the final unshard (each core adds bo * 0.5 so the host sum
restores bo once).

Device pipeline per core (fp16 matmuls at N=1024 -> full PE rate + FWL;
PSUM accumulation, softmax, attention probabilities and outputs in fp32):
  1. Projections: QT/KT in [dk, tokens] fp16 layout (lhsT = W tile), V in
     [tokens, dk] fp16 layout interleaved per head with a ones column so
     the P@V matmul also produces softmax row sums for free.
  2. Per head, two q-half sweeps: phase A computes S^T = K^T.T@Q^T per
     k-tile (fp32 PSUM), exp on ACT (fp16 out), and accumulates ctx^T
     (plus sums row) in PSUM; softmax row sums are transposed to column
     layout with tiny PE transposes and reciprocated.  Phase B recomputes
     S in natural [q, k] layout, exp to fp32, and DVE scales rows by the
     reciprocal sums to produce attention rows, DMAd out.
  3. ctx^T tiles are normalized once at the end (reciprocal rows broadcast
     back from a DRAM spill) and projected through Wo rows.
"""

import os
import sys

import numpy as np

for _p in ("/root/.axon_site/_ro/trn_rl_repo", "/opt/trn_rl_repo"):
    if os.path.isdir(_p) and _p not in sys.path:
        sys.path.append(_p)

import concourse.bacc as bacc_mod
import concourse.bass as bass
import concourse.mybir as mybir
from concourse.bass_utils import run_bass_kernel_spmd
from concourse.tile import TileContext

B, L, D, H, DK = 4, 2048, 1024, 16, 64
NCORES = 8
HPC = H // 2  # heads per core
DHC = HPC * DK  # proj columns per core (512)
P = 128
VW = HPC * (DK + 1)  # V tile width incl. ones columns (520)
FP32 = mybir.dt.float32
FP16 = mybir.dt.float16
AF = mybir.ActivationFunctionType
ALU = mybir.AluOpType

N_KT = D // P  # 8   k-tiles over d_model in projections
N_MT = DHC // P  # 4   dk-col tiles of QT/KT
N_TT = L // P  # 16  token tiles
NCH = 512  # matmul moving chunk (one fp32 PSUM bank)
HCH = 1024  # q/k half width

LAST_RESULT = {}


def _ensure_ntff_hook():
    """Register the axon NTFF profile hook if the antenv stub lacks it."""
    import types

    try:
        import antenv.axon_hooks  # noqa: F401

        return
    except ImportError:
        pass
    m = types.ModuleType("antenv.axon_hooks")
    hook_box = [None]
    m.set_axon_ntff_profile_hook = lambda h: hook_box.__setitem__(0, h)
    m.get_axon_ntff_profile_hook = lambda: hook_box[0]
    sys.modules["antenv.axon_hooks"] = m
    import antenv

    antenv.axon_hooks = m
    try:
        from trn_agent_boot.trn_boot import _ntff_profile_via_ctypes

        hook = _ntff_profile_via_ctypes("/opt/axon/libaxon_pjrt.so")
        if hook is not None:
            m.set_axon_ntff_profile_hook(hook)
    except Exception as e:
        print("ntff hook setup failed:", e)


def _build() -> bass.Bass:
    nc = bacc_mod.Bacc()

    xq_t = nc.dram_tensor("xq_t", [D, L], FP16, kind="ExternalInput")
    xk_t = nc.dram_tensor("xk_t", [D, L], FP16, kind="ExternalInput")
    xv_t = nc.dram_tensor("xv_t", [D, L], FP16, kind="ExternalInput")
    wq = nc.dram_tensor("wq", [D, DHC], FP16, kind="ExternalInput")
    wk = nc.dram_tensor("wk", [D, DHC], FP16, kind="ExternalInput")
    wv = nc.dram_tensor("wv", [D, DHC], FP16, kind="ExternalInput")
    wo = nc.dram_tensor("wo", [DHC, D], FP16, kind="ExternalInput")
    bq = nc.dram_tensor("bq", [1, DHC], FP32, kind="ExternalInput")
    bk = nc.dram_tensor("bk", [1, DHC], FP32, kind="ExternalInput")
    bv = nc.dram_tensor("bv", [1, DHC], FP32, kind="ExternalInput")
    bo = nc.dram_tensor("bo", [1, D], FP32, kind="ExternalInput")
    attn_out = nc.dram_tensor("attn_out", [HPC, L, L], FP32, kind="ExternalOutput")
    out_part = nc.dram_tensor("out_part", [L, D], FP32, kind="ExternalOutput")
    sum_spill = nc.dram_tensor("sum_spill", [HPC, L], FP32)

    with TileContext(nc) as tc:
        with (
            tc.tile_pool(name="qt", bufs=N_MT) as qt_pool,
            tc.tile_pool(name="kt", bufs=N_MT) as kt_pool,
            tc.tile_pool(name="vt", bufs=N_TT) as vt_pool,
            tc.tile_pool(name="cxt", bufs=N_MT) as cxt_pool,
            tc.tile_pool(name="small", bufs=1) as small_pool,
            tc.tile_pool(name="hsm", bufs=2) as hsm_pool,
            tc.tile_pool(name="ps", bufs=3, space="PSUM") as ps_pool,
            tc.tile_pool(name="ctx", bufs=1, space="PSUM") as ctx_pool,
        ):
            neg_col = small_pool.tile([P, 1], FP32)
            nc.vector.memset(neg_col, -1.0)

            # per-partition bias columns for QT/KT: col m holds b[m*128:(m+1)*128]
            bq_cols = small_pool.tile([P, N_MT], FP32)
            bk_cols = small_pool.tile([P, N_MT], FP32)
            nc.sync.dma_start(bq_cols, bq.rearrange("1 (m p) -> p m", p=P))
            nc.sync.dma_start(bk_cols, bk.rearrange("1 (m p) -> p m", p=P))
            bv_bcast = small_pool.tile([P, DHC], FP32)
            _src = bv[:, :]
            nc.gpsimd.dma_start(
                bv_bcast,
                bass.AP(tensor=_src.tensor, offset=_src.offset, ap=[[0, P], [1, DHC]]),
            )
            bo_bcast = small_pool.tile([P, D], FP32)
            _src = bo[:, :]
            nc.gpsimd.dma_start(
                bo_bcast,
                bass.AP(tensor=_src.tensor, offset=_src.offset, ap=[[0, P], [1, D]]),
            )

            qt_tiles = [
                qt_pool.tile([P, L], FP16, name=f"qt{i}", tag="qt")
                for i in range(N_MT)
            ]
            kt_tiles = [
                kt_pool.tile([P, L], FP16, name=f"kt{i}", tag="kt")
                for i in range(N_MT)
            ]
            vt_tiles = [
                vt_pool.tile([P, VW], FP16, name=f"vt{i}", tag="vt")
                for i in range(N_TT)
            ]

            # ---------------- projections ----------------
            with (
                tc.tile_pool(name="xt", bufs=N_KT + 1) as xt_pool,
                tc.tile_pool(name="w", bufs=N_KT + 1) as w_pool,
            ):
                for x_dram, w_dram, mode in (
                    (xq_t, wq, "QT"),
                    (xk_t, wk, "KT"),
                    (xv_t, wv, "V"),
                ):
                    xts = []
                    ws = []
                    for kt in range(N_KT):
                        xtile = xt_pool.tile(
                            [P, L], FP16, name=f"x_{mode}{kt}", tag="xt"
                        )
                        nc.sync.dma_start(xtile, x_dram[kt * P : (kt + 1) * P, :])
                        xts.append(xtile)
                        wtile = w_pool.tile(
                            [P, DHC], FP16, name=f"w_{mode}{kt}", tag="w"
                        )
                        nc.sync.dma_start(wtile, w_dram[kt * P : (kt + 1) * P, :])
                        ws.append(wtile)

                    if mode in ("QT", "KT"):
                        dst = qt_tiles if mode == "QT" else kt_tiles
                        bcols = bq_cols if mode == "QT" else bk_cols
                        for m in range(N_MT):
                            for q in range(L // NCH):
                                ps = ps_pool.tile([P, NCH], FP32, name="ps", tag="s")
                                for kt in range(N_KT):
                                    nc.tensor.matmul(
                                        ps,
                                        lhsT=ws[kt][:, m * P : (m + 1) * P],
                                        rhs=xts[kt][:, q * NCH : (q + 1) * NCH],
                                        start=(kt == 0),
                                        stop=(kt == N_KT - 1),
                                    )
                                nc.vector.tensor_scalar_add(
                                    dst[m][:, q * NCH : (q + 1) * NCH],
                                    ps,
                                    bcols[:, m : m + 1],
                                )
                    else:
                        for mt in range(N_TT):
                            ps = ps_pool.tile([P, DHC], FP32, name="ps", tag="s")
                            for kt in range(N_KT):
                                nc.tensor.matmul(
                                    ps,
                                    lhsT=xts[kt][:, mt * P : (mt + 1) * P],
                                    rhs=ws[kt],
                                    start=(kt == 0),
                                    stop=(kt == N_KT - 1),
                                )
                            vt = vt_tiles[mt]
                            vt_r = vt.rearrange("p (h c) -> p h c", c=DK + 1)
                            nc.vector.memset(vt_r[:, :, DK : DK + 1], 1.0)
                            nc.vector.tensor_tensor(
                                vt_r[:, :, 0:DK],
                                ps.rearrange("p (h c) -> p h c", c=DK),
                                bv_bcast.rearrange("p (h c) -> p h c", c=DK),
                                op=ALU.add,
                            )

            # ---------------- attention ----------------
            cxt_tiles = [
                cxt_pool.tile([P, L], FP16, name=f"cxt{i}", tag="cxt")
                for i in range(N_MT)
            ]
            with (
                tc.tile_pool(name="pt", bufs=4) as pt_pool,
                tc.tile_pool(name="stg", bufs=2) as stg_pool,
                tc.tile_pool(name="attn", bufs=5) as attn_pool,
                tc.tile_pool(name="wo", bufs=N_MT) as wo_pool,
                tc.tile_pool(name="out", bufs=2) as out_pool,
            ):
                wo_tiles = []
                for ct in range(N_MT):
                    wtile = wo_pool.tile([P, D], FP16, name=f"wo{ct}", tag="wo")
                    nc.sync.dma_start(wtile, wo[ct * P : (ct + 1) * P, :])
                    wo_tiles.append(wtile)

                for h in range(HPC):
                    th, r0 = h // 2, (h % 2) * DK
                    qth = qt_tiles[th][r0 : r0 + DK, :]
                    kth = kt_tiles[th][r0 : r0 + DK, :]

                    # ---- phase A: S^T tiles, exp, ctx^T (+sums) accumulation
                    nlc = hsm_pool.tile([P, N_TT], FP32, name="nlc", tag="nlc")
                    for half in range(2):
                        q0 = half * HCH
                        ctx = ctx_pool.tile([P, HCH], FP32, name="ctx", tag="ctx")
                        for kt in range(N_TT):
                            vh = vt_tiles[kt][:, h * (DK + 1) : (h + 1) * (DK + 1)]
                            st = ps_pool.tile([P, HCH], FP32, name="st", tag="s")
                            for c in range(2):
                                nc.tensor.matmul(
                                    st[:, c * NCH : (c + 1) * NCH],
                                    lhsT=kth[:, kt * P : (kt + 1) * P],
                                    rhs=qth[:, q0 + c * NCH : q0 + (c + 1) * NCH],
                                    start=True,
                                    stop=True,
                                )
                            pt = pt_pool.tile([P, HCH], FP16, name="pt", tag="pt")
                            nc.scalar.activation(pt, st, AF.Exp, scale=0.125)
                            for c in range(2):
                                nc.tensor.matmul(
                                    ctx[0 : DK + 1, c * NCH : (c + 1) * NCH],
                                    lhsT=vh,
                                    rhs=pt[:, c * NCH : (c + 1) * NCH],
                                    start=(kt == 0),
                                    stop=(kt == N_TT - 1),
                                )

                        # stage ctx out of PSUM fast; recip cols via PE transposes
                        stg = stg_pool.tile([P, HCH], FP32, name="stg", tag="stg")
                        nc.vector.tensor_copy(
                            stg[0 : DK + 1, :], ctx[0 : DK + 1, :]
                        )
                        nc.sync.dma_start(
                            sum_spill[h : h + 1, q0 : q0 + HCH],
                            stg[DK : DK + 1, :],
                        )
                        lnr = hsm_pool.tile([1, HCH], FP32, name="lnr", tag="lnr")
                        nc.scalar.activation(lnr, stg[DK : DK + 1, :], AF.Ln)
                        scp = ctx_pool.tile([P, 8], FP32, name="scp", tag="ctx")
                        for j in range(8):
                            nc.tensor.transpose(
                                scp[:, j : j + 1],
                                lnr[0:1, j * P : (j + 1) * P],
                                neg_col[0:1, 0:1],
                            )
                        nc.vector.tensor_scalar_mul(
                            nlc[:, half * 8 : (half + 1) * 8], scp, -1.0
                        )
                        # unnormalized fp16 ctx^T copy releases stg
                        nc.vector.tensor_copy(
                            cxt_tiles[th][r0 : r0 + DK, q0 : q0 + HCH],
                            stg[0:DK, :],
                        )

                    # ---- phase B: natural S, exp fp32, row-scale, DMA out
                    for qt in range(N_TT):
                        at = attn_pool.tile([P, L], FP32, name="at", tag="at")
                        for half in range(2):
                            st = ps_pool.tile([P, HCH], FP32, name="st", tag="s")
                            for c in range(2):
                                ks = half * HCH + c * NCH
                                nc.tensor.matmul(
                                    st[:, c * NCH : (c + 1) * NCH],
                                    lhsT=qth[:, qt * P : (qt + 1) * P],
                                    rhs=kth[:, ks : ks + NCH],
                                    start=True,
                                    stop=True,
                                )
                            nc.scalar.activation(
                                at[:, half * HCH : (half + 1) * HCH],
                                st,
                                AF.Exp,
                                bias=nlc[:, qt : qt + 1],
                                scale=0.125,
                            )
                        nc.sync.dma_start(attn_out[h, qt * P : (qt + 1) * P, :], at)

                # ---- normalize ctx^T (all heads) now that recips are spilled
                for ct in range(N_MT):
                    recb = hsm_pool.tile([P, L], FP32, name="recb", tag="recb")
                    for sub in range(2):
                        hh = 2 * ct + sub
                        _src = sum_spill[hh : hh + 1, :]
                        nc.gpsimd.dma_start(
                            recb[sub * DK : (sub + 1) * DK, :],
                            bass.AP(
                                tensor=_src.tensor,
                                offset=_src.offset,
                                ap=[[0, DK], [1, L]],
                            ),
                        )
                    nc.vector.reciprocal(recb, recb)
                    nc.vector.tensor_tensor(
                        cxt_tiles[ct], cxt_tiles[ct], recb, op=ALU.mult
                    )

                # ---------------- output projection ----------------
                for qt in range(N_TT):
                    ot = out_pool.tile([P, D], FP32, name="ot", tag="ot")
                    ps = ps_pool.tile([P, D], FP32, name="ps", tag="s")
                    for n2 in range(2):
                        for ct in range(N_MT):
                            nc.tensor.matmul(
                                ps[:, n2 * NCH : (n2 + 1) * NCH],
                                lhsT=cxt_tiles[ct][:, qt * P : (qt + 1) * P],
                                rhs=wo_tiles[ct][:, n2 * NCH : (n2 + 1) * NCH],
                                start=(ct == 0),
                                stop=(ct == N_MT - 1),
                            )
                    nc.vector.tensor_add(ot, ps, bo_bcast)
                    nc.sync.dma_start(out_part[qt * P : (qt + 1) * P, :], ot)

    nc.compile()
    return nc


_NC_CACHE = []


def kernel(query, key_, value, Wq, bq, Wk, bk, Wv, bv, Wo, bo, trace=False):
    query = np.asarray(query, np.float32)
    key_ = np.asarray(key_, np.float32)
    value = np.asarray(value, np.float32)
    Wq, Wk, Wv, Wo = (np.asarray(a, np.float32) for a in (Wq, Wk, Wv, Wo))
    bq, bk, bv, bo = (np.asarray(a, np.float32) for a in (bq, bk, bv, bo))

    if not _NC_CACHE:
        _NC_CACHE.append(_build())
    nc = _NC_CACHE[0]

    in_maps = []
    for c in range(NCORES):
        b, s = c // 2, (c % 2) * DHC
        in_maps.append(
            {
                "xq_t": np.ascontiguousarray(query[b].T.astype(np.float16)),
                "xk_t": np.ascontiguousarray(key_[b].T.astype(np.float16)),
                "xv_t": np.ascontiguousarray(value[b].T.astype(np.float16)),
                "wq": np.ascontiguousarray(Wq[:, s : s + DHC].astype(np.float16)),
                "wk": np.ascontiguousarray(Wk[:, s : s + DHC].astype(np.float16)),
                "wv": np.ascontiguousarray(Wv[:, s : s + DHC].astype(np.float16)),
                "wo": np.ascontiguousarray(Wo[s : s + DHC, :].astype(np.float16)),
                "bq": np.ascontiguousarray(bq.reshape(1, -1)[:, s : s + DHC]),
                "bk": np.ascontiguousarray(bk.reshape(1, -1)[:, s : s + DHC]),
                "bv": np.ascontiguousarray(bv.reshape(1, -1)[:, s : s + DHC]),
                "bo": np.ascontiguousarray(bo.reshape(1, -1)) * 0.5,
            }
        )

    if trace:
        _ensure_ntff_hook()
    res = run_bass_kernel_spmd(nc, in_maps, list(range(NCORES)), trace=trace)
    LAST_RESULT["exec_time_ns"] = res.exec_time_ns
    LAST_RESULT["trace"] = res.instructions_and_trace

    attn = np.empty((B, H, L, L), np.float32)
    out = np.empty((B, L, D), np.float32)
    for c in range(NCORES):
        b, h0 = c // 2, (c % 2) * HPC
        attn[b, h0 : h0 + HPC] = res.results[c]["attn_out"]
    for b in range(B):
        out[b] = res.results[2 * b]["out_part"] + res.results[2 * b + 1]["out_part"]
    return out, attn
